# revision 1
# baseline (speedup 1.0000x reference)
"""Trainium2 Bass kernel for a prototypical-network classification head.

Computes, for each of 512 independent tasks:
    prototypes = class-means of support vectors  (5 classes x 5 shots, D=1600)
    logits     = -scale * (||q||^2 - 2 q.p + ||p||^2) / D      (75 queries)

Sharding: pure data parallel, 64 tasks per NeuronCore across 8 cores.

Per-core plan (all static shapes, fp32):
  Phase A : load support slab (1600 rows x 1600), one-hot block-diag matmuls
            compute PT[d, task*5+c] = 2 * prototype^T directly (transpose +
            scatter-mean fused in a single PE pass over S).
  Phase A2: ACT squares of PT + ones-column matmul burst -> -BB row (1, 320).
  Phase B : per 128-query global tile: DMA, PE transpose of 13 D-chunks into
            PSUM, ACT copies -> SBUF Q^T, fused square+reduce -> AA column,
            tiny PE transpose -> AA row.  Per task: 13 accumulating matmuls
            (2P^T)^T @ Q^T plus two K=1 matmuls injecting -AA and -BB into
            the same PSUM accumulation -> psum = 2AB - AA - BB.
  Output  : logits^T gathered globally, PE transpose back to (q, 5),
            tensor_scalar multiply by scale/D, DMA out.
"""

import numpy as np

TASKS = 512
N_WAY = 5
N_SHOT = 5
N_QUERY = 75
D = 1600
N_SUPPORT = N_WAY * N_SHOT
N_CORES = 8
TPC = TASKS // N_CORES            # tasks per core = 64
QPC = TPC * N_QUERY               # queries per core = 4800
SPC = TPC * N_SUPPORT             # support rows per core = 1600

P = 128                           # partitions
NCHUNK = (D + P - 1) // P         # 13 D-chunks (12x128 + 64)
DCS = [min(P, D - P * k) for k in range(NCHUNK)]
NQT = (QPC + P - 1) // P          # 38 query tiles (37x128 + 64)
QTS = [min(P, QPC - P * j) for j in range(NQT)]
GSIZE = 5                         # tasks per support group
NGRP = (TPC + GSIZE - 1) // GSIZE # 13 groups (12x5 + 4)
GTASKS = [min(GSIZE, TPC - GSIZE * g) for g in range(NGRP)]
GROWS = [t * N_SUPPORT for t in GTASKS]  # 125 / 100 rows

_COMPILED = None


def _build_nc():
    import os
    import concourse.bacc as bacc
    import concourse.mybir as mybir
    import concourse.tile as tile

    stage = int(os.environ.get("KSTAGE", "99"))
    AA_MODE = os.environ.get("AA_MODE", "act")

    f32 = mybir.dt.float32
    nc = bacc.Bacc("TRN2", debug=False, num_devices=N_CORES)

    q_dram = nc.dram_tensor("q", (QPC, D), f32, kind="ExternalInput")
    s_dram = nc.dram_tensor("s", (SPC, D), f32, kind="ExternalInput")
    w_dram = nc.dram_tensor("w", (GSIZE * N_SUPPORT, NGRP, GSIZE * N_WAY), f32,
                            kind="ExternalInput")
    ident_dram = nc.dram_tensor("ident", (P, P), f32, kind="ExternalInput")
    aux_dram = nc.dram_tensor("aux", (4, P), f32, kind="ExternalInput")
    bbcol_dram = nc.dram_tensor("bbcol", (P, 1), f32, kind="ExternalInput")
    scolv_dram = nc.dram_tensor("scolv", (P, 1), f32, kind="ExternalInput")
    out_dram = nc.dram_tensor("out", (QPC, N_WAY), f32, kind="ExternalOutput")

    PTW = TPC * N_WAY             # 320 prototype columns

    with tile.TileContext(nc) as tc:
        with (
            tc.tile_pool(name="sb", bufs=1) as sb,
            tc.tile_pool(name="ps", bufs=1, space="PSUM") as ps,
        ):
            # ---- constants ----
            ident = sb.tile([P, P], f32, tag="ident", bufs=1)
            nc.sync.dma_start(ident[:], ident_dram.ap())
            ones_r = sb.tile([1, P], f32, tag="ones_r", bufs=1)
            nc.sync.dma_start(ones_r[:], aux_dram.ap()[0:1, :])
            neg_r = sb.tile([1, P], f32, tag="neg_r", bufs=1)
            nc.sync.dma_start(neg_r[:], aux_dram.ap()[1:2, :])
            bbcol = sb.tile([P, 1], f32, tag="bbcol", bufs=1)
            nc.sync.dma_start(bbcol[:], bbcol_dram.ap())
            w_sb = sb.tile([GSIZE * N_SUPPORT, NGRP, GSIZE * N_WAY], f32,
                           tag="w", bufs=1)
            nc.sync.dma_start(w_sb[:], w_dram.ap())

            scol = sb.tile([P, 1], f32, tag="scol", bufs=1)
            nc.sync.dma_start(scol[:], scolv_dram.ap())

            # ---- phase A: PT[d, 5t+c] = 2 * prototype^T ----
            pt = sb.tile([P, NCHUNK, PTW], f32, tag="pt", bufs=1)
            for g in range(NGRP):
                st = sb.tile([GSIZE * N_SUPPORT, D], f32, tag="sn", bufs=2)
                nc.sync.dma_start(st[0:GROWS[g], :],
                                  s_dram.ap()[GSIZE * N_SUPPORT * g:
                                              GSIZE * N_SUPPORT * g + GROWS[g], :])
                nw = N_WAY * GTASKS[g]
                for k4 in range((NCHUNK + 3) // 4):
                    hi = min(NCHUNK, 4 * k4 + 4)
                    ptp = ps.tile([P, 4, N_WAY * GSIZE], f32, tag="big", bufs=5)
                    for k in range(4 * k4, hi):
                        nc.tensor.matmul(
                            ptp[0:DCS[k], k - 4 * k4, 0:nw],
                            st[0:GROWS[g], P * k:P * k + DCS[k]],
                            w_sb[0:GROWS[g], g, 0:nw],
                            start=(k == 4 * k4), stop=(k == hi - 1),
                        )
                    pmax = DCS[4 * k4]
                    nc.scalar.copy(
                        pt[0:pmax, 4 * k4:hi, N_WAY * GSIZE * g:
                           N_WAY * GSIZE * g + nw],
                        ptp[0:pmax, 0:hi - 4 * k4, 0:nw],
                    )

            # ---- phase A2: -BB row ----
            bb_ps = ps.tile([1, PTW], f32, tag="misc", bufs=1)
            for k in range(NCHUNK):
                p2 = sb.tile([P, PTW], f32, tag="p2", bufs=2)
                nc.scalar.square(p2[0:DCS[k], :], pt[0:DCS[k], k, :])
                nc.tensor.matmul(bb_ps[:], bbcol[0:DCS[k], :], p2[0:DCS[k], :],
                                 start=(k == 0), stop=(k == NCHUNK - 1))
            bbrow = sb.tile([1, PTW], f32, tag="bbrow", bufs=1)
            nc.vector.tensor_copy(bbrow[:], bb_ps[:])

            # ---- phase B ----
            ltg = sb.tile([N_WAY, QPC], f32, tag="ltg", bufs=1)
            aarow = sb.tile([1, QPC], f32, tag="aarow", bufs=1)
            qt_tiles = [None] * NQT
            tasks_done = 0
            tiles_out = 0

            for j in range(NQT):
                if stage < 2:
                    break
                n_q = QTS[j]
                qn = sb.tile([P, D], f32, tag="qn", bufs=3)
                nc.sync.dma_start(qn[0:n_q, :],
                                  q_dram.ap()[P * j:P * j + n_q, :])

                # transpose 13 D-chunks into PSUM (4 chunks per bank)
                qt = sb.tile([P, NCHUNK, P], f32, tag="qt", bufs=3)
                qt_tiles[j] = qt
                for k4 in range((NCHUNK + 3) // 4):
                    tp = ps.tile([P, 512], f32, tag="big", bufs=5)
                    hi = min(NCHUNK, 4 * k4 + 4)
                    for k in range(4 * k4, hi):
                        nc.tensor.transpose(
                            tp[0:DCS[k], P * (k - 4 * k4):
                               P * (k - 4 * k4) + n_q],
                            qn[0:n_q, P * k:P * k + DCS[k]],
                            ident[0:n_q, 0:n_q],
                        )
                    width = P * (hi - 4 * k4)
                    pmax = DCS[4 * k4]
                    nc.scalar.copy(
                        qt[0:pmax, 4 * k4:hi, 0:n_q],
                        tp[:, 0:width].rearrange(
                            "p (a b) -> p a b", b=P)[0:pmax, :, 0:n_q],
                    )

                # AA = sum_d q^2 (alternate engines), then transpose to a row
                if stage < 3:
                    continue
                aac = sb.tile([P, 1], f32, tag="aac", bufs=2)
                sq = sb.tile([P, D], f32, tag="sq", bufs=2)
                if AA_MODE == "ttr":
                    nc.vector.tensor_tensor_reduce(
                        out=sq[0:n_q, :], in0=qn[0:n_q, :], in1=qn[0:n_q, :],
                        scale=1.0, scalar=0.0,
                        op0=mybir.AluOpType.mult, op1=mybir.AluOpType.add,
                        accum_out=aac[0:n_q, :],
                    )
                else:
                    nc.scalar.activation(
                        sq[0:n_q, :], qn[0:n_q, :],
                        mybir.ActivationFunctionType.Square,
                        accum_out=aac[0:n_q, :],
                    )
                aat_ps = ps.tile([1, P], f32, tag="misc", bufs=1)
                nc.tensor.matmul(aat_ps[0:1, 0:n_q], aac[0:n_q, :],
                                 ident[0:n_q, 0:n_q], start=True, stop=True)
                nc.vector.tensor_copy(aarow[0:1, P * j:P * j + n_q],
                                      aat_ps[0:1, 0:n_q])

                # main matmuls for tasks fully covered by tiles <= j
                if stage < 4:
                    continue
                hi_q = P * j + n_q
                while tasks_done < TPC and \
                        N_QUERY * (tasks_done + 1) <= hi_q:
                    t = tasks_done
                    q0 = N_QUERY * t
                    j0 = q0 // P
                    j1 = (q0 + N_QUERY - 1) // P
                    mp = ps.tile([N_WAY, N_QUERY], f32, tag="main", bufs=2)
                    for k in range(NCHUNK):
                        lhs = pt[0:DCS[k], k, N_WAY * t:N_WAY * t + N_WAY]
                        if j0 == j1:
                            o = q0 - P * j0
                            nc.tensor.matmul(
                                mp[:, 0:N_QUERY],
                                lhs,
                                qt_tiles[j0][0:DCS[k], k, o:o + N_QUERY],
                                start=(k == 0), stop=False,
                            )
                        else:
                            o = q0 - P * j0
                            la = P - o
                            nc.tensor.matmul(
                                mp[:, 0:la],
                                lhs,
                                qt_tiles[j0][0:DCS[k], k, o:P],
                                start=(k == 0), stop=False,
                            )
                            nc.tensor.matmul(
                                mp[:, la:N_QUERY],
                                lhs,
                                qt_tiles[j1][0:DCS[k], k, 0:N_QUERY - la],
                                start=False, stop=False,
                            )
                    # inject -AA and -BB into the same accumulation
                    nc.tensor.matmul(mp[:], neg_r[0:1, 0:N_WAY],
                                     aarow[0:1, q0:q0 + N_QUERY],
                                     start=False, stop=False)
                    nc.tensor.matmul(mp[:], bbrow[0:1, N_WAY * t:N_WAY * t + N_WAY],
                                     ones_r[0:1, 0:N_QUERY],
                                     start=False, stop=True)
                    nc.vector.tensor_copy(ltg[:, q0:q0 + N_QUERY], mp[:])
                    tasks_done += 1

                # emit finished output tiles
                if stage < 5:
                    continue
                done_q = N_QUERY * tasks_done
                while tiles_out < NQT and \
                        P * tiles_out + QTS[tiles_out] <= done_q:
                    jj = tiles_out
                    n_o = QTS[jj]
                    ln_ps = ps.tile([P, N_WAY], f32, tag="misc", bufs=1)
                    nc.tensor.matmul(ln_ps[0:n_o, :],
                                     ltg[:, P * jj:P * jj + n_o],
                                     ident[0:N_WAY, 0:N_WAY],
                                     start=True, stop=True)
                    ln = sb.tile([P, N_WAY], f32, tag="ln", bufs=3)
                    nc.vector.tensor_scalar(
                        out=ln[0:n_o, :], in0=ln_ps[0:n_o, :],
                        scalar1=scol[0:n_o, :], scalar2=None,
                        op0=mybir.AluOpType.mult,
                    )
                    nc.sync.dma_start(out_dram.ap()[P * jj:P * jj + n_o, :],
                                      ln[0:n_o, :])
                    tiles_out += 1

    nc.compile()
    return nc


def _get_compiled():
    global _COMPILED
    if _COMPILED is None:
        _COMPILED = _build_nc()
    return _COMPILED


def _make_in_maps(inputs):
    return _build_in_maps(
        inputs["query"], inputs["support"], inputs["support_labels"],
        inputs["scale"])


def _build_in_maps(query, support, support_labels, scale):
    query = np.asarray(query, dtype=np.float32)
    support = np.asarray(support, dtype=np.float32)
    support_labels = np.asarray(support_labels)
    scale_np = np.asarray(scale, dtype=np.float32).reshape(1, 1)

    ident = np.eye(P, dtype=np.float32)
    aux = np.zeros((4, P), dtype=np.float32)
    aux[0, :] = 1.0
    aux[1, :] = -1.0
    aux[2, :] = 1.0 / D
    bbcol = np.full((P, 1), -0.25, dtype=np.float32)

    in_maps = []
    for c in range(N_CORES):
        t0 = TPC * c
        q_slab = np.ascontiguousarray(
            query[t0:t0 + TPC].reshape(QPC, D))
        s_slab = np.ascontiguousarray(
            support[t0:t0 + TPC].reshape(SPC, D))
        labels = support_labels[t0:t0 + TPC]
        # per-(group, task) one-hot weights: 2 * oh / count
        w = np.zeros((GSIZE * N_SUPPORT, NGRP, GSIZE * N_WAY), dtype=np.float32)
        for g in range(NGRP):
            for tl in range(GTASKS[g]):
                t = GSIZE * g + tl
                oh = (labels[t][:, None] ==
                      np.arange(N_WAY)[None, :]).astype(np.float32)
                counts = oh.sum(axis=0, keepdims=True)
                w[N_SUPPORT * tl:N_SUPPORT * (tl + 1), g,
                  N_WAY * tl:N_WAY * (tl + 1)] = 2.0 * oh / counts
        in_maps.append({
            "q": q_slab, "s": s_slab, "w": w, "ident": ident,
            "aux": aux, "bbcol": bbcol,
            "scolv": np.full((P, 1), scale_np.ravel()[0] / D, np.float32),
        })
    return in_maps


def kernel(query, support, support_labels, scale, n_way, n_shot):
    from concourse import bass_utils

    nc = _get_compiled()
    in_maps = _build_in_maps(query, support, support_labels, scale)
    res = bass_utils.run_bass_kernel_spmd(nc, in_maps, core_ids=list(range(N_CORES)))
    out = np.concatenate(
        [res.results[c]["out"].reshape(TPC, N_QUERY, N_WAY)
         for c in range(N_CORES)], axis=0)
    return out



# revision 3
# speedup vs baseline: 2.6806x; 2.6806x over previous
"""Trainium2 Bass kernel for a prototypical-network classification head.

Computes, for each of 512 independent tasks:
    prototypes = class-means of support vectors  (5 classes x 5 shots, D=1600)
    logits     = -scale * (||q||^2 - 2 q.p + ||p||^2) / D      (75 queries)

Sharding: pure data parallel, 64 tasks per NeuronCore across 8 cores.

End-to-end latency here is dominated by host->device transfer of the
inputs (the cores are tunneled), so query/support are shipped as
float8_e4m3 (4x fewer bytes than fp32) and upcast to bf16 on device.
The tolerance budget (rel 2e-2) comfortably absorbs the quantization:
measured ~1.0e-2 on the reference inputs.

Per-core device plan (all static shapes):
  Phase A : load support slab (fp8), cast to bf16, one-hot block-diag
            matmuls -> PSUM fp32; copy out with scale 2/n_shot so
            PT[d, task*5+c] = 2 * prototype^T in bf16.
  Phase A2: ACT squares of PT (fp32) + (-1/4)-column fp32 matmul burst
            -> -BB row (1, 320) fp32.
  Phase B : per 128-query tile: DMA fp8, cast to bf16, PE transpose of
            13 D-chunks into PSUM (bf16), DVE copies -> SBUF Q^T bf16,
            ACT square+reduce -> AA column fp32, small fp32 matmul ->
            AA row.  Per task: 13 accumulating bf16 matmuls
            (2P^T)^T @ Q^T plus two K=1 fp32 matmuls injecting -AA and
            -BB into the same PSUM accumulation -> psum = 2AB - AA - BB.
  Output  : logits^T gathered, PE transpose back to (q, 5) in fp32,
            tensor_scalar multiply by scale/D, DMA out as fp16.
"""

import numpy as np
import ml_dtypes

TASKS = 512
N_WAY = 5
N_SHOT = 5
N_QUERY = 75
D = 1600
N_SUPPORT = N_WAY * N_SHOT
N_CORES = 8
TPC = TASKS // N_CORES            # tasks per core = 64
QPC = TPC * N_QUERY               # queries per core = 4800
SPC = TPC * N_SUPPORT             # support rows per core = 1600

P = 128                           # partitions
NCHUNK = (D + P - 1) // P         # 13 D-chunks (12x128 + 64)
DCS = [min(P, D - P * k) for k in range(NCHUNK)]
NQT = (QPC + P - 1) // P          # 38 query tiles (37x128 + 64)
QTS = [min(P, QPC - P * j) for j in range(NQT)]
GSIZE = 5                         # tasks per support group
NGRP = (TPC + GSIZE - 1) // GSIZE # 13 groups (12x5 + 4)
GTASKS = [min(GSIZE, TPC - GSIZE * g) for g in range(NGRP)]
GROWS = [t * N_SUPPORT for t in GTASKS]  # 125 / 100 rows

F8 = ml_dtypes.float8_e4m3
BF16 = ml_dtypes.bfloat16

_COMPILED = None


def _build_nc():
    import concourse.bacc as bacc
    import concourse.mybir as mybir
    import concourse.tile as tile

    f32 = mybir.dt.float32
    f16 = mybir.dt.float16
    bf16 = mybir.dt.bfloat16
    f8 = mybir.dt.float8e4
    nc = bacc.Bacc("TRN2", debug=False, num_devices=N_CORES)

    q_dram = nc.dram_tensor("q", (QPC, D), f8, kind="ExternalInput")
    s_dram = nc.dram_tensor("s", (SPC, D), f8, kind="ExternalInput")
    w_dram = nc.dram_tensor("w", (GSIZE * N_SUPPORT, NGRP, GSIZE * N_WAY), bf16,
                            kind="ExternalInput")
    identb_dram = nc.dram_tensor("identb", (P, P), bf16, kind="ExternalInput")
    ident_dram = nc.dram_tensor("ident", (P, P), f32, kind="ExternalInput")
    aux_dram = nc.dram_tensor("aux", (4, P), f32, kind="ExternalInput")
    bbcol_dram = nc.dram_tensor("bbcol", (P, 1), f32, kind="ExternalInput")
    scolv_dram = nc.dram_tensor("scolv", (P, 1), f32, kind="ExternalInput")
    ptsc_dram = nc.dram_tensor("ptsc", (P, 1), f32, kind="ExternalInput")
    out_dram = nc.dram_tensor("out", (QPC, N_WAY), f16, kind="ExternalOutput")

    PTW = TPC * N_WAY             # 320 prototype columns

    with tile.TileContext(nc) as tc:
        with (
            tc.tile_pool(name="sb", bufs=1) as sb,
            tc.tile_pool(name="ps", bufs=1, space="PSUM") as ps,
        ):
            # ---- constants ----
            identb = sb.tile([P, P], bf16, tag="identb", bufs=1)
            nc.sync.dma_start(identb[:], identb_dram.ap())
            ident = sb.tile([P, P], f32, tag="ident", bufs=1)
            nc.sync.dma_start(ident[:], ident_dram.ap())
            ones_r = sb.tile([1, P], f32, tag="ones_r", bufs=1)
            nc.sync.dma_start(ones_r[:], aux_dram.ap()[0:1, :])
            neg_r = sb.tile([1, P], f32, tag="neg_r", bufs=1)
            nc.sync.dma_start(neg_r[:], aux_dram.ap()[1:2, :])
            bbcol = sb.tile([P, 1], f32, tag="bbcol", bufs=1)
            nc.sync.dma_start(bbcol[:], bbcol_dram.ap())
            w_sb = sb.tile([GSIZE * N_SUPPORT, NGRP, GSIZE * N_WAY], bf16,
                           tag="w", bufs=1)
            nc.sync.dma_start(w_sb[:], w_dram.ap())
            scol = sb.tile([P, 1], f32, tag="scol", bufs=1)
            nc.sync.dma_start(scol[:], scolv_dram.ap())
            ptsc = sb.tile([P, 1], f32, tag="ptsc", bufs=1)
            nc.sync.dma_start(ptsc[:], ptsc_dram.ap())

            # ---- phase A: PT[d, 5t+c] = 2 * prototype^T (bf16) ----
            pt = sb.tile([P, NCHUNK, PTW], bf16, tag="pt", bufs=1)
            for g in range(NGRP):
                st8 = sb.tile([GSIZE * N_SUPPORT, D], f8, tag="s8", bufs=2)
                nc.sync.dma_start(st8[0:GROWS[g], :],
                                  s_dram.ap()[GSIZE * N_SUPPORT * g:
                                              GSIZE * N_SUPPORT * g + GROWS[g], :])
                st = sb.tile([GSIZE * N_SUPPORT, D], bf16, tag="s16", bufs=2)
                nc.scalar.copy(st[0:GROWS[g], :], st8[0:GROWS[g], :])
                nw = N_WAY * GTASKS[g]
                for k4 in range((NCHUNK + 3) // 4):
                    hi = min(NCHUNK, 4 * k4 + 4)
                    ptp = ps.tile([P, 4, N_WAY * GSIZE], f32, tag="bigf", bufs=2)
                    for k in range(4 * k4, hi):
                        nc.tensor.matmul(
                            ptp[0:DCS[k], k - 4 * k4, 0:nw],
                            st[0:GROWS[g], P * k:P * k + DCS[k]],
                            w_sb[0:GROWS[g], g, 0:nw],
                            start=(k == 4 * k4), stop=(k == hi - 1),
                        )
                    pmax = DCS[4 * k4]
                    nc.scalar.activation(
                        pt[0:pmax, 4 * k4:hi, N_WAY * GSIZE * g:
                           N_WAY * GSIZE * g + nw],
                        ptp[0:pmax, 0:hi - 4 * k4, 0:nw],
                        mybir.ActivationFunctionType.Copy,
                        scale=ptsc[0:pmax, :],
                    )

            # ---- phase A2: -BB row (fp32) ----
            bb_ps = ps.tile([1, PTW], f32, tag="misc", bufs=1)
            for k in range(NCHUNK):
                p2 = sb.tile([P, PTW], f32, tag="p2", bufs=2)
                nc.scalar.square(p2[0:DCS[k], :], pt[0:DCS[k], k, :])
                nc.tensor.matmul(bb_ps[:], bbcol[0:DCS[k], :], p2[0:DCS[k], :],
                                 start=(k == 0), stop=(k == NCHUNK - 1))
            bbrow = sb.tile([1, PTW], f32, tag="bbrow", bufs=1)
            nc.vector.tensor_copy(bbrow[:], bb_ps[:])

            # ---- phase B ----
            ltg = sb.tile([N_WAY, QPC], f32, tag="ltg", bufs=1)
            aarow = sb.tile([1, QPC], f32, tag="aarow", bufs=1)
            qt_tiles = [None] * NQT
            tasks_done = 0
            tiles_out = 0

            for j in range(NQT):
                n_q = QTS[j]
                qn8 = sb.tile([P, D], f8, tag="q8", bufs=3)
                nc.sync.dma_start(qn8[0:n_q, :],
                                  q_dram.ap()[P * j:P * j + n_q, :])
                qn = sb.tile([P, D], bf16, tag="q16", bufs=2)
                nc.scalar.copy(qn[0:n_q, :], qn8[0:n_q, :])

                # transpose 13 D-chunks into PSUM (4 chunks per bank)
                qt = sb.tile([P, NCHUNK, P], bf16, tag="qt", bufs=3)
                qt_tiles[j] = qt
                for k4 in range((NCHUNK + 3) // 4):
                    tp = ps.tile([P, 512], bf16, tag="bigt", bufs=3)
                    hi = min(NCHUNK, 4 * k4 + 4)
                    for k in range(4 * k4, hi):
                        nc.tensor.transpose(
                            tp[0:DCS[k], P * (k - 4 * k4):
                               P * (k - 4 * k4) + n_q],
                            qn[0:n_q, P * k:P * k + DCS[k]],
                            identb[0:n_q, 0:n_q],
                        )
                    width = P * (hi - 4 * k4)
                    pmax = DCS[4 * k4]
                    nc.vector.tensor_copy(
                        qt[0:pmax, 4 * k4:hi, 0:n_q],
                        tp[:, 0:width].rearrange(
                            "p (a b) -> p a b", b=P)[0:pmax, :, 0:n_q],
                    )

                # AA = sum_d q^2 (fp32), then transpose to a row
                aac = sb.tile([P, 1], f32, tag="aac", bufs=2)
                sq = sb.tile([P, D], f32, tag="sq", bufs=2)
                nc.scalar.activation(
                    sq[0:n_q, :], qn[0:n_q, :],
                    mybir.ActivationFunctionType.Square,
                    accum_out=aac[0:n_q, :],
                )
                aat_ps = ps.tile([1, P], f32, tag="misc", bufs=1)
                nc.tensor.matmul(aat_ps[0:1, 0:n_q], aac[0:n_q, :],
                                 ident[0:n_q, 0:n_q], start=True, stop=True)
                nc.vector.tensor_copy(aarow[0:1, P * j:P * j + n_q],
                                      aat_ps[0:1, 0:n_q])

                # main matmuls for tasks fully covered by tiles <= j
                hi_q = P * j + n_q
                while tasks_done < TPC and \
                        N_QUERY * (tasks_done + 1) <= hi_q:
                    t = tasks_done
                    q0 = N_QUERY * t
                    j0 = q0 // P
                    j1 = (q0 + N_QUERY - 1) // P
                    mp = ps.tile([N_WAY, N_QUERY], f32, tag="main", bufs=2)
                    for k in range(NCHUNK):
                        lhs = pt[0:DCS[k], k, N_WAY * t:N_WAY * t + N_WAY]
                        if j0 == j1:
                            o = q0 - P * j0
                            nc.tensor.matmul(
                                mp[:, 0:N_QUERY],
                                lhs,
                                qt_tiles[j0][0:DCS[k], k, o:o + N_QUERY],
                                start=(k == 0), stop=False,
                            )
                        else:
                            o = q0 - P * j0
                            la = P - o
                            nc.tensor.matmul(
                                mp[:, 0:la],
                                lhs,
                                qt_tiles[j0][0:DCS[k], k, o:P],
                                start=(k == 0), stop=False,
                            )
                            nc.tensor.matmul(
                                mp[:, la:N_QUERY],
                                lhs,
                                qt_tiles[j1][0:DCS[k], k, 0:N_QUERY - la],
                                start=False, stop=False,
                            )
                    # inject -AA and -BB into the same accumulation (fp32)
                    nc.tensor.matmul(mp[:], neg_r[0:1, 0:N_WAY],
                                     aarow[0:1, q0:q0 + N_QUERY],
                                     start=False, stop=False)
                    nc.tensor.matmul(mp[:], bbrow[0:1, N_WAY * t:N_WAY * t + N_WAY],
                                     ones_r[0:1, 0:N_QUERY],
                                     start=False, stop=True)
                    nc.vector.tensor_copy(ltg[:, q0:q0 + N_QUERY], mp[:])
                    tasks_done += 1

                # emit finished output tiles
                done_q = N_QUERY * tasks_done
                while tiles_out < NQT and \
                        P * tiles_out + QTS[tiles_out] <= done_q:
                    jj = tiles_out
                    n_o = QTS[jj]
                    ln_ps = ps.tile([P, N_WAY], f32, tag="misc", bufs=1)
                    nc.tensor.matmul(ln_ps[0:n_o, :],
                                     ltg[:, P * jj:P * jj + n_o],
                                     ident[0:N_WAY, 0:N_WAY],
                                     start=True, stop=True)
                    ln = sb.tile([P, N_WAY], f16, tag="ln", bufs=3)
                    nc.vector.tensor_scalar(
                        out=ln[0:n_o, :], in0=ln_ps[0:n_o, :],
                        scalar1=scol[0:n_o, :], scalar2=None,
                        op0=mybir.AluOpType.mult,
                    )
                    nc.sync.dma_start(out_dram.ap()[P * jj:P * jj + n_o, :],
                                      ln[0:n_o, :])
                    tiles_out += 1

    nc.compile()
    return nc


def _get_compiled():
    global _COMPILED
    if _COMPILED is None:
        _COMPILED = _build_nc()
    return _COMPILED


def _to_f8(x):
    """fp32 -> float8_e4m3, via torch when available (faster on one core)."""
    try:
        import torch
        t = torch.from_numpy(np.ascontiguousarray(x))
        return t.to(torch.float8_e4m3fn).view(torch.uint8).numpy().view(F8)
    except Exception:
        return x.astype(F8)


def _make_in_maps(inputs):
    return _build_in_maps(
        inputs["query"], inputs["support"], inputs["support_labels"],
        inputs["scale"])


def _build_in_maps(query, support, support_labels, scale):
    query = np.asarray(query, dtype=np.float32).reshape(TASKS, N_QUERY, D)
    support = np.asarray(support, dtype=np.float32).reshape(TASKS, N_SUPPORT, D)
    support_labels = np.asarray(support_labels).reshape(TASKS, N_SUPPORT)
    scale_np = np.asarray(scale, dtype=np.float32).reshape(-1)

    q8 = _to_f8(query).reshape(TASKS * N_QUERY, D)
    s8 = _to_f8(support).reshape(TASKS * N_SUPPORT, D)

    identb = np.eye(P, dtype=BF16)
    ident = np.eye(P, dtype=np.float32)
    aux = np.zeros((4, P), dtype=np.float32)
    aux[0, :] = 1.0
    aux[1, :] = -1.0
    bbcol = np.full((P, 1), -0.25, dtype=np.float32)
    scolv = np.full((P, 1), scale_np[0] / D, np.float32)

    # one-hot counts; when balanced (the reference setup), ship a pure 0/1
    # one-hot (exact in bf16) and fold 2/count into the on-device PT copy.
    oh = (support_labels[..., None] ==
          np.arange(N_WAY)[None, None, :])                  # (T, S, C) bool
    counts = oh.sum(axis=1)                                 # (T, C)
    uniform = (counts == counts.ravel()[0]).all() and counts.ravel()[0] > 0
    if uniform:
        ptsc = np.full((P, 1), 2.0 / float(counts.ravel()[0]), np.float32)
        wf = oh.astype(np.float32)
    else:
        ptsc = np.ones((P, 1), np.float32)
        wf = 2.0 * oh.astype(np.float32) / np.maximum(counts, 1)[:, None, :]

    in_maps = []
    for c in range(N_CORES):
        t0 = TPC * c
        # per-(group, task) block-diagonal one-hot weights
        w = np.zeros((GSIZE * N_SUPPORT, NGRP, GSIZE * N_WAY), dtype=BF16)
        for g in range(NGRP):
            for tl in range(GTASKS[g]):
                t = GSIZE * g + tl
                w[N_SUPPORT * tl:N_SUPPORT * (tl + 1), g,
                  N_WAY * tl:N_WAY * (tl + 1)] = wf[t0 + t].astype(BF16)
        in_maps.append({
            "q": q8[QPC * c:QPC * (c + 1)],
            "s": s8[SPC * c:SPC * (c + 1)],
            "w": w, "identb": identb, "ident": ident,
            "aux": aux, "bbcol": bbcol, "scolv": scolv, "ptsc": ptsc,
        })
    return in_maps


def kernel(query, support, support_labels, scale, n_way, n_shot):
    from concourse import bass_utils

    nc = _get_compiled()
    in_maps = _build_in_maps(query, support, support_labels, scale)
    res = bass_utils.run_bass_kernel_spmd(nc, in_maps, core_ids=list(range(N_CORES)))
    out = np.concatenate(
        [res.results[c]["out"].astype(np.float32).reshape(TPC, N_QUERY, N_WAY)
         for c in range(N_CORES)], axis=0)
    return out
